# revision 4
# baseline (speedup 1.0000x reference)
"""Multi-head causal attention (dense_transformer) on 8 trn2 NeuronCores.

Problem: x[4, 2048, 768], 12 heads of d_head=64, causal softmax, out proj.

Sharding: data-parallel over batch (4) x tensor-parallel over heads
(2 groups of 6). Core c handles (batch c//2, heads 6*(c%2)..6*(c%2)+5) and
returns its partial output sum over its heads; the host adds the two
partials per batch ("all-reduce" of size 2 done host-side).

v2 layout/schedule (vs v1):
  - Scores matmuls are ROW-TILED: heads 2p / 2p+1 contract over PE array
    rows 0-63 / 64-127 concurrently (tile_position auto-derived from the
    base partitions), so a head-pair's scores cost one head's matmul
    time. QT/KT/zT are pair-packed [128, 2048] (even head rows 0-63, odd
    head rows 64-127) with no zero-padding needed.
  - The schedule interleaves projection / output-projection matmuls as
    "ballast" into the exp-bound attention phase so the PE never idles
    while ACT chews through the 120 exps. Ballast items carry a deadline
    chunk index; all chunk-j projection items are force-drained before
    chunk-j attention is emitted (PE queues execute in order, so a
    later-emitted projection a score matmul depends on would deadlock).
  - xT is DMA'd q-chunk 0 first so the first projections start as soon
    as possible; weights stream next, then the rest of x.
  - PSUM: scores pool 2x[128,1024] (4 banks) + z pool 2x[65,512]
    (2 banks) + proj pool 2x[128,512] (2 banks) = 8 banks.
  - exp: softmax without max-subtraction (scores are O(1) here; exp
    cannot overflow): P = exp(s/8), denominator accumulated via a
    constant-1 65th column of V in the PV matmul. Normalization:
    reciprocal read straight from PSUM -> DRAM hop -> partition-broadcast
    DMA -> multiply into zT. All PSUM->SBUF copies run on DVE, keeping
    ACT exclusively for exp.

Biases: b_K cancels in softmax; b_V/b_O fold into a constant row added
host-side; b_Q is always zero (falls back to numpy otherwise).
"""
import os
import sys
from collections import deque

sys.path.insert(0, "/opt/trn_rl_repo")

import numpy as np
import ml_dtypes

D_MODEL, N_HEADS, D_HEAD = 768, 12, 64
BATCH, SEQ = 4, 2048
HPG = 6           # heads per group (per core)
NPAIR = HPG // 2  # head pairs per core
NCORES = 8
QC = 512          # q chunk (moving operand width)
KT_TILES = SEQ // 128
QC_TILES = SEQ // QC
MT = D_MODEL // 128  # contraction tiles for projections
BF16 = ml_dtypes.bfloat16
FAR = 10 ** 9     # "no deadline" for ballast items

_prog_cache = {}


def _numpy_ref(normalized_resid_pre, W_Q, W_K, W_V, W_O, b_Q, b_K, b_V, b_O):
    x = normalized_resid_pre.astype(np.float32)
    Q = np.einsum("bsm,hmd->bshd", x, W_Q) + b_Q
    K = np.einsum("bsm,hmd->bshd", x, W_K) + b_K
    V = np.einsum("bsm,hmd->bshd", x, W_V) + b_V
    scores = np.einsum("bqhd,bkhd->bhqk", Q, K) / np.sqrt(np.float32(W_Q.shape[-1]))
    s = x.shape[1]
    causal = np.tril(np.ones((s, s), dtype=bool))
    scores = np.where(causal, scores, -np.inf)
    scores -= scores.max(axis=-1, keepdims=True)
    e = np.exp(scores)
    probs = e / e.sum(axis=-1, keepdims=True)
    z = np.einsum("bkhd,bhqk->bqhd", V, probs)
    return (np.einsum("bqhd,hdm->bqm", z, W_O) + b_O).astype(np.float32)


def _build_program():
    from concourse import bacc, tile
    import concourse.bass as bass
    import concourse.mybir as mybir

    f32 = mybir.dt.float32
    bf16 = mybir.dt.bfloat16

    nc = bacc.Bacc(None)
    xT_d = nc.dram_tensor("xT", [D_MODEL, SEQ], bf16, kind="ExternalInput")
    wq_d = nc.dram_tensor("wq", [D_MODEL, HPG * D_HEAD], bf16, kind="ExternalInput")
    wk_d = nc.dram_tensor("wk", [D_MODEL, HPG * D_HEAD], bf16, kind="ExternalInput")
    wv_d = nc.dram_tensor("wv", [D_MODEL, HPG * D_HEAD], bf16, kind="ExternalInput")
    wo_d = nc.dram_tensor("wo", [HPG * D_HEAD, D_MODEL], bf16, kind="ExternalInput")
    mask_d = nc.dram_tensor("mask", [128, 128], bf16, kind="ExternalInput")
    out_d = nc.dram_tensor("out", [SEQ, D_MODEL], f32, kind="ExternalOutput")
    recip_d = nc.dram_tensor("recip_scratch", [HPG * QC_TILES, QC], f32)

    with tile.TileContext(nc) as tc:
        with (
            tc.tile_pool(name="persist", bufs=1) as persist,
            tc.tile_pool(name="expsb", bufs=6) as expsb,
            tc.tile_pool(name="rbsb", bufs=4) as rbsb,
            tc.tile_pool(name="rtsb", bufs=4) as rtsb,
            tc.tile_pool(name="outsb", bufs=3) as outsb,
            tc.tile_pool(name="ps_ss", bufs=2, space="PSUM") as ps_ss,
            tc.tile_pool(name="ps_z", bufs=2, space="PSUM") as ps_z,
            tc.tile_pool(name="ps_pj", bufs=2, space="PSUM") as ps_pj,
        ):
            # ---- persistent SBUF tiles ----
            xT = [persist.tile([128, SEQ], bf16, tag=f"xT{i}", name=f"xT{i}") for i in range(MT)]
            wq = [persist.tile([128, HPG * D_HEAD], bf16, tag=f"wq{i}", name=f"wq{i}") for i in range(MT)]
            wk = [persist.tile([128, HPG * D_HEAD], bf16, tag=f"wk{i}", name=f"wk{i}") for i in range(MT)]
            wv = [persist.tile([128, HPG * D_HEAD], bf16, tag=f"wv{i}", name=f"wv{i}") for i in range(MT)]
            wo = [persist.tile([128, D_MODEL], bf16, tag=f"wo{i}", name=f"wo{i}") for i in range(NPAIR)]
            # pair-packed: rows 0-63 head 2p, rows 64-127 head 2p+1
            QT = [persist.tile([128, SEQ], bf16, tag=f"QT{p}", name=f"QT{p}") for p in range(NPAIR)]
            KT = [persist.tile([128, SEQ], bf16, tag=f"KT{p}", name=f"KT{p}") for p in range(NPAIR)]
            zT = [persist.tile([128, SEQ], bf16, tag=f"zT{p}", name=f"zT{p}") for p in range(NPAIR)]
            V = [persist.tile([128, HPG, D_HEAD + 1], bf16, tag=f"V{i}", name=f"V{i}") for i in range(KT_TILES)]
            mask01 = persist.tile([128, 128], bf16, tag="mask01")

            # ---- input DMAs, ordered for fast start ----
            nc.sync.dma_start(out=mask01, in_=mask_d[:, :])
            cs0 = slice(0, QC)
            for i in range(MT):
                nc.sync.dma_start(out=xT[i][:, cs0], in_=xT_d[128 * i : 128 * (i + 1), cs0])
            for i in range(MT):
                nc.sync.dma_start(out=wq[i], in_=wq_d[128 * i : 128 * (i + 1), :])
            for i in range(MT):
                nc.sync.dma_start(out=wk[i], in_=wk_d[128 * i : 128 * (i + 1), :])
            for i in range(MT):
                nc.sync.dma_start(out=wv[i], in_=wv_d[128 * i : 128 * (i + 1), :])
            for c in range(1, QC_TILES):
                cs = slice(QC * c, QC * (c + 1))
                for i in range(MT):
                    nc.sync.dma_start(out=xT[i][:, cs], in_=xT_d[128 * i : 128 * (i + 1), cs])
            for p in range(NPAIR):
                nc.sync.dma_start(out=wo[p], in_=wo_d[128 * p : 128 * (p + 1), :])
            # constant-1 denominator column of V (gpsimd: off the DVE queue)
            for kt in range(KT_TILES):
                nc.gpsimd.memset(V[kt][:, :, D_HEAD : D_HEAD + 1], 1.0)
            # dummy exp so the ACT table load (~1.3us) happens during input DMA
            warm = persist.tile([1, 1], f32, tag="warm")
            nc.vector.memset(warm, 0.0)
            nc.scalar.activation(out=warm, in_=warm,
                                 func=mybir.ActivationFunctionType.Exp, scale=1.0)

            # ---- ballast machinery ----
            # items: (cost_ns, deadline_chunk, fn). All items with deadline
            # <= j must be emitted before chunk-j attention is emitted.
            ballast = deque()

            def drain(budget):
                while ballast and budget > 0:
                    cost, _, fn = ballast.popleft()
                    fn()
                    budget -= cost

            def force_chunk(j):
                while any(d <= j for _, d, _ in ballast):
                    ballast.popleft()[2]()

            def drain_all():
                while ballast:
                    ballast.popleft()[2]()

            # ---- projection groups (each -> two ~0.6us ballast items) ----
            def emit_qk(dst, w, p, c):
                cols = slice(128 * p, 128 * (p + 1))
                qs = slice(QC * c, QC * (c + 1))
                cell = []

                def first():
                    ps = ps_pj.tile([128, QC], f32, tag="pj", name="psqk")
                    cell.append(ps)
                    for m in range(3):
                        nc.tensor.matmul(ps, lhsT=w[m][:, cols], rhs=xT[m][:, qs],
                                         start=(m == 0), stop=False,
                                         skip_group_check=True)

                def second():
                    ps = cell[0]
                    for m in range(3, MT):
                        nc.tensor.matmul(ps, lhsT=w[m][:, cols], rhs=xT[m][:, qs],
                                         start=False, stop=(m == MT - 1),
                                         skip_group_check=True)
                    nc.vector.tensor_copy(dst[p][:, qs], ps)

                return [(660, c, first), (660, c, second)]

            def emit_v(kt):
                ks = slice(128 * kt, 128 * (kt + 1))
                c = kt // 4
                cell = []

                def first():
                    ps = ps_pj.tile([128, HPG * D_HEAD], f32, tag="pj", name="psv")
                    cell.append(ps)
                    for m in range(3):
                        nc.tensor.matmul(ps, lhsT=xT[m][:, ks], rhs=wv[m],
                                         start=(m == 0), stop=False,
                                         skip_group_check=True)

                def second():
                    ps = cell[0]
                    for m in range(3, MT):
                        nc.tensor.matmul(ps, lhsT=xT[m][:, ks], rhs=wv[m],
                                         start=False, stop=(m == MT - 1),
                                         skip_group_check=True)
                    nc.vector.tensor_copy(
                        V[kt][:, :, 0:D_HEAD],
                        ps.rearrange("p (h d) -> p h d", h=HPG))

                return [(500, c, first), (500, c, second)]

            def proj_chunk_units(c):
                units = []
                for p in range(NPAIR):
                    units += emit_qk(QT, wq, p, c)
                    units += emit_qk(KT, wk, p, c)
                for kt in range(4 * c, 4 * (c + 1)):
                    units += emit_v(kt)
                return units

            # ---- output projection (per 128-row tile of the output) ----
            def outproj_ctile(c):
                cs = slice(128 * c, 128 * (c + 1))
                cell = []

                def mk(p):
                    def fn():
                        if p == 0:
                            cell.append(ps_pj.tile([128, QC], f32, tag="pj", name="pso_a"))
                            cell.append(ps_pj.tile([128, D_MODEL - QC], f32, tag="pj", name="pso_b"))
                        pso_a, pso_b = cell
                        nc.tensor.matmul(pso_a, lhsT=zT[p][:, cs], rhs=wo[p][:, 0:QC],
                                         start=(p == 0), stop=(p == NPAIR - 1),
                                         skip_group_check=True)
                        nc.tensor.matmul(pso_b, lhsT=zT[p][:, cs], rhs=wo[p][:, QC:D_MODEL],
                                         start=(p == 0), stop=(p == NPAIR - 1),
                                         skip_group_check=True)
                    return fn

                def fin():
                    pso_a, pso_b = cell
                    outt = outsb.tile([128, D_MODEL], f32, tag="out", name="outt")
                    nc.vector.tensor_copy(outt[:, 0:QC], pso_a)
                    nc.vector.tensor_copy(outt[:, QC:D_MODEL], pso_b)
                    nc.sync.dma_start(out=out_d[cs, :], in_=outt)

                return [(320, FAR, mk(0)), (320, FAR, mk(1)), (320, FAR, mk(2)),
                        (250, FAR, fin)]

            def outproj_units(j):
                units = []
                for c in range(4 * j, 4 * (j + 1)):
                    units += outproj_ctile(c)
                return units

            # ---- attention ----
            def emit_scores_pair(p, j, kt2):
                # row-tiled: head 2p contracts on PE rows 0-63, head 2p+1 on
                # rows 64-127; the two matmuls run concurrently.
                ssA = ps_ss.tile([128, 2 * QC], f32, tag="ss", name="ssA")
                ssB = ps_ss.tile([128, 2 * QC], f32, tag="ss", name="ssB")
                off0 = 0
                for u in (0, 1):
                    kt = kt2 + u
                    delta = kt - 4 * j
                    off = 128 * delta if delta >= 0 else 0
                    if u == 0:
                        off0 = off
                    ks = slice(128 * kt, 128 * (kt + 1))
                    qs = slice(QC * j + off, QC * (j + 1))
                    for rows, ss in ((slice(0, 64), ssA), (slice(64, 128), ssB)):
                        nc.tensor.matmul(
                            ss[:, QC * u + off : QC * (u + 1)],
                            lhsT=KT[p][rows, ks],
                            rhs=QT[p][rows, qs],
                            start=True, stop=True,
                            skip_group_check=True,
                        )
                expA = expsb.tile([128, 2 * QC], bf16, tag="exp", name="expA")
                expB = expsb.tile([128, 2 * QC], bf16, tag="exp", name="expB")
                for ss, ex in ((ssA, expA), (ssB, expB)):
                    nc.scalar.activation(out=ex[:, off0:], in_=ss[:, off0:],
                                         func=mybir.ActivationFunctionType.Exp,
                                         scale=0.125)
                for u in (0, 1):
                    delta = kt2 + u - 4 * j
                    if delta >= 0:
                        blk = slice(QC * u + 128 * delta, QC * u + 128 * delta + 128)
                        nc.vector.tensor_mul(expA[:, blk], expA[:, blk], mask01)
                        nc.vector.tensor_mul(expB[:, blk], expB[:, blk], mask01)
                return expA, expB

            def emit_pv(p, j, kt2, expA, expB, zA, zB, nkt):
                for u in (0, 1):
                    kt = kt2 + u
                    delta = kt - 4 * j
                    off = 128 * delta if delta >= 0 else 0
                    for h, ex, z in ((2 * p, expA, zA), (2 * p + 1, expB, zB)):
                        nc.tensor.matmul(
                            z[:, off:QC],
                            lhsT=V[kt][:, h, :],
                            rhs=ex[:, QC * u + off : QC * (u + 1)],
                            start=(kt == 0), stop=(kt == nkt - 1),
                            skip_group_check=True,
                        )

            def emit_norm(p, j, zA, zB):
                qs = slice(QC * j, QC * (j + 1))
                for h, psz, r0 in ((2 * p, zA, 0), (2 * p + 1, zB, 64)):
                    row = HPG * j + (h % HPG)
                    # reciprocal_approx_* is a custom DVE op: PSUM input reads
                    # garbage (verified on HW), so stage the denominator row
                    # through SBUF first.
                    dtmp = rtsb.tile([1, QC], f32, tag="dt", name="dtmp")
                    nc.vector.tensor_copy(dtmp, psz[D_HEAD : D_HEAD + 1, :])
                    rtmp = rtsb.tile([1, QC], f32, tag="rt", name="rtmp")
                    nc.vector.reciprocal_approx_fast(rtmp, dtmp)
                    nc.sync.dma_start(out=recip_d[row : row + 1, :], in_=rtmp)
                    nc.vector.tensor_copy(zT[p][r0 : r0 + 64, qs], psz[0:D_HEAD, :])
                    sl = recip_d[row : row + 1, :]
                    rb = rbsb.tile([128, QC], f32, tag="rb", name="rb")
                    nc.sync.dma_start(
                        out=rb[r0 : r0 + 64, :],
                        in_=bass.AP(tensor=sl.tensor, offset=sl.offset,
                                    ap=[[0, D_HEAD]] + list(sl.ap[-1:])))
                    nc.vector.tensor_mul(zT[p][r0 : r0 + 64, qs],
                                         zT[p][r0 : r0 + 64, qs],
                                         rb[r0 : r0 + 64, :])

            def attention_unit(p, j):
                nkt = 4 * j + 4
                zA = ps_z.tile([D_HEAD + 1, QC], f32, tag="z", name="zA")
                zB = ps_z.tile([D_HEAD + 1, QC], f32, tag="z", name="zB")
                pend = None
                for kt2 in range(0, nkt, 2):
                    cur = (kt2, emit_scores_pair(p, j, kt2))
                    if pend is not None:
                        kt2p, (eA, eB) = pend
                        emit_pv(p, j, kt2p, eA, eB, zA, zB, nkt)
                    pend = cur
                    drain(700)
                drain(1000)
                kt2p, (eA, eB) = pend
                emit_pv(p, j, kt2p, eA, eB, zA, zB, nkt)
                emit_norm(p, j, zA, zB)
                drain(1500)

            # ---- main schedule ----
            for _, _, fn in proj_chunk_units(0):  # eagerly; PE chases the DMAs
                fn()
            ballast.extend(proj_chunk_units(1))
            for j in range(QC_TILES):
                force_chunk(j)
                for p in range(NPAIR):
                    attention_unit(p, j)
                if j + 2 < QC_TILES:
                    ballast.extend(proj_chunk_units(j + 2))
                ballast.extend(outproj_units(j))
            drain_all()

    nc.finalize()
    return nc


def kernel(**inputs):
    x = inputs["normalized_resid_pre"]
    W_Q, W_K, W_V, W_O = inputs["W_Q"], inputs["W_K"], inputs["W_V"], inputs["W_O"]
    b_Q, b_K, b_V, b_O = inputs["b_Q"], inputs["b_K"], inputs["b_V"], inputs["b_O"]

    expected = (
        x.shape == (BATCH, SEQ, D_MODEL)
        and W_Q.shape == (N_HEADS, D_MODEL, D_HEAD)
        and W_K.shape == (N_HEADS, D_MODEL, D_HEAD)
        and W_V.shape == (N_HEADS, D_MODEL, D_HEAD)
        and W_O.shape == (N_HEADS, D_HEAD, D_MODEL)
        and not np.any(b_Q)
    )
    if not expected:
        return _numpy_ref(**inputs)

    from concourse.bass_utils import run_bass_kernel_spmd

    if "nc" not in _prog_cache:
        _prog_cache["nc"] = _build_program()
    nc = _prog_cache["nc"]

    # host-side prep: transpose + cast + pack per head-group
    xT = np.ascontiguousarray(x.transpose(0, 2, 1)).astype(BF16)  # [B, 768, 2048]
    # b_K shifts every score in a softmax row equally -> cancels exactly.
    groups = []
    for g in range(2):
        hs = slice(HPG * g, HPG * (g + 1))
        groups.append({
            "wq": np.ascontiguousarray(W_Q[hs].transpose(1, 0, 2).reshape(D_MODEL, HPG * D_HEAD)).astype(BF16),
            "wk": np.ascontiguousarray(W_K[hs].transpose(1, 0, 2).reshape(D_MODEL, HPG * D_HEAD)).astype(BF16),
            "wv": np.ascontiguousarray(W_V[hs].transpose(1, 0, 2).reshape(D_MODEL, HPG * D_HEAD)).astype(BF16),
            "wo": np.ascontiguousarray(W_O[hs].reshape(HPG * D_HEAD, D_MODEL)).astype(BF16),
        })
    ii, jj = np.arange(128)[:, None], np.arange(128)[None, :]
    mask = np.where(jj >= ii, np.float32(1.0), np.float32(0.0)).astype(BF16)

    in_maps = []
    for c in range(NCORES):
        b, g = c // 2, c % 2
        m = {"xT": xT[b], "mask": mask}
        m.update(groups[g])
        in_maps.append(m)

    trace = bool(os.environ.get("ATTN_KERNEL_TRACE"))
    res = run_bass_kernel_spmd(nc, in_maps, list(range(NCORES)), trace=trace)
    _prog_cache["last_exec_time_ns"] = res.exec_time_ns
    _prog_cache["last_results"] = res

    # b_V/b_O fold into a constant row (softmax weights sum to 1).
    const_row = np.einsum("hd,hdm->m", b_V.astype(np.float64), W_O.astype(np.float64))
    const_row = (const_row + b_O.astype(np.float64)).astype(np.float32)

    out = np.empty((BATCH, SEQ, D_MODEL), dtype=np.float32)
    for b in range(BATCH):
        out[b] = res.results[2 * b]["out"] + res.results[2 * b + 1]["out"] + const_row
    return out
